# revision 1
# baseline (speedup 1.0000x reference)
"""Trainium2 Bass kernel for QANet-style Context-Query attention.

Problem shapes (hardcoded): B=64, C=1024, Q=128, H=512, fp32.
  S[b,c,q] = x_context[b,c,:].W1 + x_query[b,q,:].W0 + (x_query[b,q,:]*W2).x_context[b,c,:] + bias
  c2q = softmax_q(S) @ x_query                  -> [B,C,H]
  q2c = softmax_q(S) @ (softmax_c(S)^T @ x_context)  -> [B,C,H]

Sharding: data-parallel over batch, 8 batches per core on 8 NeuronCores.

Device algorithm per batch (all matmuls float32r: full PE rate, ~fp22 precision):
  - xcT = transpose(xc) via TensorE (32 [128,128] transposes)
  - S^T[q,c] accumulated in PSUM from 4 K-tiles of xqW2T.T @ xcT, plus two
    augmented K=1 matmuls adding sub1[c] (row, via M=1 matmuls of W1.T @ xcT)
    and sub0[q]+bias (column, transposed to a row).
  - E^T = exp(S^T) on ACT; accum_out gives rc[q] = sum_c E.
  - E (c-partitioned) via 8 more PE transposes; rq[c] = sum_q E via one DVE
    3D reduce.  Softmax divisions are folded into per-partition scales of the
    PSUM->SBUF copies after the combine matmuls (g-factors cancel).
  - c2q_tile = (E^T_tile.T @ xq) * (1/rq); tmp = (E.T-tiles @ xc) * (1/rc);
    q2c_tile = (E^T_tile.T @ tmp) * (1/rq).

Masks are all-ones for this problem (fill: ones) and are mathematically
no-ops; they are not shipped to the device.
"""

import sys

if "/opt/trn_rl_repo" not in sys.path:
    sys.path.insert(0, "/opt/trn_rl_repo")

from contextlib import ExitStack

import numpy as np

import concourse.bass as bass
import concourse.tile as tile
from concourse import bacc, mybir
from concourse.bass_utils import run_bass_kernel_spmd
from concourse.masks import make_identity

F32 = mybir.dt.float32
F32R = mybir.dt.float32r

B, C, Q, H = 64, 1024, 128, 512
N_CORES = 8
B_LOC = B // N_CORES  # batches per core
CT = C // 128  # 8 c-tiles
HT = H // 128  # 4 h-tiles (K tiles for S matmul)
NC_CHUNK = 512  # free-dim chunk for S^T (PSUM bank)
N_CHUNKS = C // NC_CHUNK  # 2


def r(ap):
    """View an fp32 AP as float32r (fp22-read) for TensorE."""
    return ap.bitcast(F32R)


def build_nc(b_loc=B_LOC, stage=99):
    nc = bacc.Bacc("TRN2", target_bir_lowering=False, debug=False)

    xc_d = nc.dram_tensor("xc", [b_loc, C, H], F32R, kind="ExternalInput").ap()
    xq_d = nc.dram_tensor("xq", [b_loc, Q, H], F32R, kind="ExternalInput").ap()
    w0_d = nc.dram_tensor("W0", [H], F32, kind="ExternalInput").ap()
    w1_d = nc.dram_tensor("W1", [H], F32R, kind="ExternalInput").ap()
    w2_d = nc.dram_tensor("W2", [H], F32, kind="ExternalInput").ap()
    bias_d = nc.dram_tensor("bias", [1], F32, kind="ExternalInput").ap()
    c2q_d = nc.dram_tensor("c2q", [b_loc, C, H], F32, kind="ExternalOutput").ap()
    q2c_d = nc.dram_tensor("q2c", [b_loc, C, H], F32, kind="ExternalOutput").ap()

    with tile.TileContext(nc) as tc, ExitStack() as ctx:
        consts = ctx.enter_context(tc.tile_pool(name="consts", bufs=1))
        xc_pool = ctx.enter_context(tc.tile_pool(name="xc", bufs=2))
        xct_pool = ctx.enter_context(tc.tile_pool(name="xct", bufs=2))
        et_pool = ctx.enter_context(tc.tile_pool(name="et", bufs=2))
        esb_pool = ctx.enter_context(tc.tile_pool(name="esb", bufs=2))
        small = ctx.enter_context(tc.tile_pool(name="small", bufs=3))
        outp = ctx.enter_context(tc.tile_pool(name="outp", bufs=6))
        ps_tr = ctx.enter_context(tc.tile_pool(name="ps_tr", bufs=2, space="PSUM"))
        ps_mm = ctx.enter_context(tc.tile_pool(name="ps_mm", bufs=2, space="PSUM"))
        ps_s = ctx.enter_context(tc.tile_pool(name="ps_s", bufs=2, space="PSUM"))
        ps_sm = ctx.enter_context(tc.tile_pool(name="ps_sm", bufs=2, space="PSUM"))

        # ---- one-time constants ----
        ident_f = consts.tile([128, 128], F32)
        make_identity(nc, ident_f)
        ident = consts.tile([128, 128], F32R)
        nc.vector.tensor_copy(ident, ident_f)
        identr = ident

        # W0/W2 broadcast across partitions (row vectors replicated)
        w0bc = consts.tile([128, H], F32)
        w2bc = consts.tile([128, H], F32)
        for t, src in ((w0bc, w0_d), (w2bc, w2_d)):
            bcast = bass.AP(tensor=src.tensor, offset=0, ap=[[0, 128], [1, H]])
            nc.gpsimd.dma_start(out=t, in_=bcast)
        # W1 as column tiles: w1col[p, k] = W1[k*128+p]
        w1col = consts.tile([128, HT], F32R)
        nc.sync.dma_start(out=w1col, in_=w1_d.rearrange("(k p) -> p k", p=128))
        bias_sb = consts.tile([1, 1], F32)
        nc.sync.dma_start(out=bias_sb, in_=bias_d.unsqueeze(0))
        ones_f = consts.tile([1, C], F32)
        nc.vector.memset(ones_f, 1.0)
        ones_lhs = consts.tile([1, 128], F32R)
        nc.vector.tensor_copy(ones_lhs, ones_f[:, :128])
        ones_rhs = consts.tile([1, C], F32R)
        nc.vector.tensor_copy(ones_rhs, ones_f)

        for b in range(b_loc):
            # ---- loads ----
            xc_t = xc_pool.tile([128, CT, H], F32R, tag="xc")
            nc.sync.dma_start(out=xc_t, in_=xc_d[b].rearrange("(t p) h -> p t h", p=128))
            xq_t = xc_pool.tile([128, H], F32R, tag="xq")
            nc.sync.dma_start(out=xq_t, in_=xq_d[b])

            # ---- xq * W2, sub0 ----
            xqw2 = small.tile([128, H], F32R, tag="xqw2")
            nc.vector.tensor_mul(xqw2, xq_t.bitcast(F32), w2bc)
            scr = small.tile([128, H], F32, tag="scr")
            sub0col_f = small.tile([128, 1], F32, tag="sub0col_f")
            nc.vector.tensor_mul(scr, xq_t.bitcast(F32), w0bc)
            nc.vector.tensor_reduce(
                sub0col_f, scr, axis=mybir.AxisListType.X, op=mybir.AluOpType.add)
            sub0col = small.tile([128, 1], F32R, tag="sub0col")
            nc.vector.tensor_copy(sub0col, sub0col_f)

            # ---- transpose xqw2 -> xqw2T [128h(k), 4, 128q] ----
            ps_a = ps_tr.tile([128, 512], F32, tag="tr")
            for k in range(HT):
                nc.tensor.transpose(
                    r(ps_a[:, 128 * k:128 * (k + 1)]),
                    xqw2[:, 128 * k:128 * (k + 1)], identr)
            xqw2t = small.tile([128, HT, 128], F32R, tag="xqw2t")
            nc.scalar.copy(xqw2t, ps_a.rearrange("p (k q) -> p k q", k=HT))

            # ---- sub0 row (+bias) ----
            ps_0 = ps_sm.tile([1, 512], F32, tag="sm")
            nc.tensor.transpose(r(ps_0[:, :128]), sub0col, identr)
            sub0brow = small.tile([1, 128], F32R, tag="sub0brow")
            nc.vector.tensor_scalar_add(sub0brow, ps_0[:, :128], bias_sb)

            # ---- transpose xc -> xcT [128h, HT, C] ----
            # grouped so each PSUM->SBUF copy writes a contiguous f32r range
            # (strided f32r destination APs hard-fault the engines)
            xct_t = xct_pool.tile([128, HT, C], F32R, tag="xct")
            for k in range(HT):
                for half in range(2):
                    ps_x = ps_tr.tile([128, 512], F32, tag="tr")
                    for i in range(4):
                        t = 4 * half + i
                        nc.tensor.transpose(
                            r(ps_x[:, 128 * i:128 * (i + 1)]),
                            xc_t[:, t, 128 * k:128 * (k + 1)], identr)
                    dst = xct_t[:, k, 512 * half:512 * (half + 1)]
                    if (k + half) % 2 == 0:
                        nc.vector.tensor_copy(dst, ps_x)
                    else:
                        nc.scalar.copy(dst, ps_x)

            if stage < 2:
                nc.sync.dma_start(out=c2q_d[b, 0:128, :], in_=xct_t[:, 0, 0:512].bitcast(F32))
                continue
            # ---- sub1 row: W1.T @ xcT ----
            sub1row = small.tile([1, C], F32R, tag="sub1row")
            for n in range(N_CHUNKS):
                ps_1 = ps_sm.tile([1, 512], F32, tag="sm")
                for k in range(HT):
                    nc.tensor.matmul(
                        ps_1, w1col[:, k:k + 1],
                        xct_t[:, k, NC_CHUNK * n:NC_CHUNK * (n + 1)],
                        start=(k == 0), stop=(k == HT - 1))
                nc.scalar.copy(sub1row[:, NC_CHUNK * n:NC_CHUNK * (n + 1)], ps_1)

            if stage < 3:
                nc.sync.dma_start(out=c2q_d[b, 0:128, :], in_=xct_t[:, 0, 0:512].bitcast(F32))
                continue
            # ---- S^T chunks + exp -> E^T; rc via accum ----
            et_t = et_pool.tile([128, C], F32R, tag="et")
            rc2 = small.tile([128, 2], F32, tag="rc2")
            for n in range(N_CHUNKS):
                sl = slice(NC_CHUNK * n, NC_CHUNK * (n + 1))
                ps_S = ps_s.tile([128, 512], F32, tag="s")
                for k in range(HT):
                    nc.tensor.matmul(
                        ps_S, xqw2t[:, k, :], xct_t[:, k, sl],
                        start=(k == 0), stop=False)
                nc.tensor.matmul(ps_S, ones_lhs, sub1row[:, sl],
                                 start=False, stop=False)
                nc.tensor.matmul(ps_S, sub0brow, ones_rhs[:, sl],
                                 start=False, stop=True)
                nc.scalar.activation(
                    et_t[:, sl], ps_S, mybir.ActivationFunctionType.Exp,
                    accum_out=rc2[:, n:n + 1])
            rcsum = small.tile([128, 1], F32, tag="rcsum")
            nc.vector.tensor_add(rcsum, rc2[:, 0:1], rc2[:, 1:2])
            rcinv = small.tile([128, 1], F32, tag="rcinv")
            nc.vector.reciprocal(rcinv, rcsum)

            if stage < 4:
                nc.sync.dma_start(out=c2q_d[b, 0:128, :], in_=et_t[:, 0:512].bitcast(F32))
                continue
            # ---- E (c-partitioned) via transposes; rq ----
            esb_t = esb_pool.tile([128, CT, 128], F32R, tag="esb")
            for n in range(N_CHUNKS):
                ps_e = ps_tr.tile([128, 512], F32, tag="tr")
                for i in range(4):
                    j = 4 * n + i
                    nc.tensor.transpose(
                        r(ps_e[:, 128 * i:128 * (i + 1)]),
                        et_t[:, 128 * j:128 * (j + 1)], identr)
                nc.vector.tensor_copy(
                    esb_t[:, 4 * n:4 * (n + 1), :],
                    ps_e.rearrange("p (j q) -> p j q", j=4))
            rq = small.tile([128, CT], F32, tag="rq")
            nc.vector.tensor_reduce(
                rq, esb_t.bitcast(F32), axis=mybir.AxisListType.X, op=mybir.AluOpType.add)
            rqinv = small.tile([128, CT], F32, tag="rqinv")
            nc.vector.reciprocal(rqinv, rq)

            if stage < 5:
                nc.sync.dma_start(out=c2q_d[b, 0:128, :], in_=esb_t[:, 0, :].bitcast(F32))
                continue
            # ---- c2q = (E^T_m.T @ xq) * rqinv_m ----
            for m in range(CT):
                ps_y = ps_mm.tile([128, 512], F32, tag="mm")
                nc.tensor.matmul(ps_y, et_t[:, 128 * m:128 * (m + 1)],
                                 xq_t, start=True, stop=True)
                o = outp.tile([128, H], F32, tag="out")
                nc.scalar.activation(o, ps_y, mybir.ActivationFunctionType.Copy,
                                     scale=rqinv[:, m:m + 1])
                nc.sync.dma_start(out=c2q_d[b, 128 * m:128 * (m + 1), :], in_=o)

            if stage < 6:
                continue
            # ---- tmp = (E.T @ xc) * rcinv ----
            ps_t0 = ps_mm.tile([128, 512], F32, tag="mm")
            for t in range(CT):
                nc.tensor.matmul(ps_t0, esb_t[:, t, :], xc_t[:, t, :],
                                 start=(t == 0), stop=(t == CT - 1))
            tmp = small.tile([128, H], F32R, tag="tmp")
            nc.scalar.activation(tmp, ps_t0, mybir.ActivationFunctionType.Copy,
                                 scale=rcinv)

            # ---- q2c = (E^T_m.T @ tmp) * rqinv_m ----
            for m in range(CT):
                ps_z = ps_mm.tile([128, 512], F32, tag="mm")
                nc.tensor.matmul(ps_z, et_t[:, 128 * m:128 * (m + 1)],
                                 tmp, start=True, stop=True)
                o = outp.tile([128, H], F32, tag="out")
                nc.vector.tensor_scalar_mul(o, ps_z, rqinv[:, m:m + 1])
                nc.sync.dma_start(out=q2c_d[b, 128 * m:128 * (m + 1), :], in_=o)

    nc.finalize()
    return nc


_CACHED_NC = None


def kernel(x_context, x_query, context_mask, query_mask, W0, W1, W2, bias):
    global _CACHED_NC
    if _CACHED_NC is None:
        _CACHED_NC = build_nc()
    nc = _CACHED_NC

    x_context = np.ascontiguousarray(x_context, dtype=np.float32)
    x_query = np.ascontiguousarray(x_query, dtype=np.float32)
    in_maps = []
    for i in range(N_CORES):
        sl = slice(i * B_LOC, (i + 1) * B_LOC)
        in_maps.append({
            "xc": x_context[sl],
            "xq": x_query[sl],
            "W0": np.asarray(W0, dtype=np.float32),
            "W1": np.asarray(W1, dtype=np.float32),
            "W2": np.asarray(W2, dtype=np.float32),
            "bias": np.asarray(bias, dtype=np.float32),
        })

    res = run_bass_kernel_spmd(nc, in_maps, core_ids=list(range(N_CORES)))
    c2q = np.concatenate([rm["c2q"] for rm in res.results], axis=0)
    q2c = np.concatenate([rm["q2c"] for rm in res.results], axis=0)
    return c2q, q2c



# revision 4
# speedup vs baseline: 1.7478x; 1.7478x over previous
"""Trainium2 Bass kernel for QANet-style Context-Query attention (bf16).

Problem shapes (hardcoded): B=64, C=1024, Q=128, H=512, fp32 I/O.
  S[b,c,q] = x_context[b,c,:].W1 + x_query[b,q,:].W0 + (x_query[b,q,:]*W2).x_context[b,c,:] + bias
  c2q = softmax_q(S) @ x_query                       -> [B,C,H]
  q2c = softmax_q(S) @ (softmax_c(S)^T @ x_context)  -> [B,C,H]

Sharding: data-parallel over batch, 8 batches per core on 8 NeuronCores.

All device I/O and SBUF residency is bf16 (host down/up-casts); PSUM
accumulation stays fp32.  rel-err budget is 2e-2; bf16 rounding costs ~5e-3.

Device algorithm per batch:
  - xqw2' = xq*W2 + W1 (folds the sub1[c] term into the K-contraction:
    sum_h (xq*W2 + W1)[q,h]*xc[c,h] = sub2[q,c] + sub1[c]).
  - sub0[q]+bias is applied as the per-partition bias of the Exp activation
    (E^T = exp(S^T_partial*1 + sub0b)), so S needs no augmentation matmuls.
  - xcT via 32 PE transposes (bf16, 1 cycle/row); S^T accumulated from 4
    K-tiles of xqw2'T.T @ xcT; exp on ACT with accum_out giving rc[q].
  - E (c-partitioned) via 8 PE transposes; rq[c] via one DVE 3D reduce.
  - Per m-tile: c2q and q2c matmuls target one [128,1024] 2-bank PSUM tile,
    evacuated by a single scaled copy (softmax divisions fold into the
    per-partition scales; g-factors cancel).  Outputs staged in SBUF and
    written with 2 DMAs per batch.

Masks are all-ones for this problem (fill: ones) and mathematically no-ops;
they are not shipped to the device.
"""

import sys

if "/opt/trn_rl_repo" not in sys.path:
    sys.path.insert(0, "/opt/trn_rl_repo")

from contextlib import ExitStack

import ml_dtypes
import numpy as np

import concourse.bass as bass
import concourse.tile as tile
from concourse import bacc, mybir
from concourse.bass_utils import run_bass_kernel_spmd
from concourse.masks import make_identity

F32 = mybir.dt.float32
BF16 = mybir.dt.bfloat16
BF16_NP = ml_dtypes.bfloat16

B, C, Q, H = 64, 1024, 128, 512
N_CORES = 8
B_LOC = B // N_CORES  # batches per core
CT = C // 128  # 8 c-tiles
HT = H // 128  # 4 h-tiles (K tiles for S matmul)
NC_CHUNK = 512  # free-dim chunk for S^T (PSUM bank)
N_CHUNKS = C // NC_CHUNK  # 2


def build_nc(b_loc=B_LOC):
    nc = bacc.Bacc("TRN2", target_bir_lowering=False, debug=False)

    xc_d = nc.dram_tensor("xc", [b_loc, C, H], BF16, kind="ExternalInput").ap()
    xq_d = nc.dram_tensor("xq", [b_loc, Q, H], BF16, kind="ExternalInput").ap()
    w0_d = nc.dram_tensor("W0", [H], BF16, kind="ExternalInput").ap()
    w1_d = nc.dram_tensor("W1", [H], BF16, kind="ExternalInput").ap()
    w2_d = nc.dram_tensor("W2", [H], BF16, kind="ExternalInput").ap()
    bias_d = nc.dram_tensor("bias", [1], F32, kind="ExternalInput").ap()
    c2q_d = nc.dram_tensor("c2q", [b_loc, C, H], BF16, kind="ExternalOutput").ap()
    q2c_d = nc.dram_tensor("q2c", [b_loc, C, H], BF16, kind="ExternalOutput").ap()

    with tile.TileContext(nc) as tc, ExitStack() as ctx:
        consts = ctx.enter_context(tc.tile_pool(name="consts", bufs=1))
        xc_pool = ctx.enter_context(tc.tile_pool(name="xc", bufs=2))
        xct_pool = ctx.enter_context(tc.tile_pool(name="xct", bufs=2))
        et_pool = ctx.enter_context(tc.tile_pool(name="et", bufs=2))
        esb_pool = ctx.enter_context(tc.tile_pool(name="esb", bufs=2))
        small = ctx.enter_context(tc.tile_pool(name="small", bufs=3))
        stage = ctx.enter_context(tc.tile_pool(name="stage", bufs=2))
        ps_tr = ctx.enter_context(tc.tile_pool(name="ps_tr", bufs=2, space="PSUM"))
        ps_s = ctx.enter_context(tc.tile_pool(name="ps_s", bufs=2, space="PSUM"))
        ps_o = ctx.enter_context(tc.tile_pool(name="ps_o", bufs=2, space="PSUM"))

        # ---- one-time constants ----
        ident = consts.tile([128, 128], BF16)
        make_identity(nc, ident)

        # W0/W1/W2 broadcast across partitions (row vectors replicated)
        w0bc = consts.tile([128, H], BF16)
        w1bc = consts.tile([128, H], BF16)
        w2bc = consts.tile([128, H], BF16)
        for t, src in ((w0bc, w0_d), (w1bc, w1_d), (w2bc, w2_d)):
            bcast = bass.AP(tensor=src.tensor, offset=0, ap=[[0, 128], [1, H]])
            nc.gpsimd.dma_start(out=t, in_=bcast)
        biascol = consts.tile([128, 1], F32)
        nc.gpsimd.dma_start(
            out=biascol,
            in_=bass.AP(tensor=bias_d.tensor, offset=0, ap=[[0, 128], [1, 1]]))

        for b in range(b_loc):
            # ---- loads ----
            xc_t = xc_pool.tile([128, CT, H], BF16, tag="xc")
            nc.sync.dma_start(out=xc_t, in_=xc_d[b].rearrange("(t p) h -> p t h", p=128))
            xq_t = xc_pool.tile([128, H], BF16, tag="xq")
            nc.sync.dma_start(out=xq_t, in_=xq_d[b])

            # ---- xqw2' = xq*W2 + W1 ; sub0 + bias ----
            xqw2 = small.tile([128, H], BF16, tag="xqw2")
            nc.vector.tensor_mul(xqw2, xq_t, w2bc)
            nc.vector.tensor_add(xqw2, xqw2, w1bc)
            scr = small.tile([128, H], F32, tag="scr")
            nc.vector.tensor_mul(scr, xq_t, w0bc)
            sub0f = small.tile([128, 1], F32, tag="sub0f")
            nc.vector.tensor_reduce(
                sub0f, scr, axis=mybir.AxisListType.X, op=mybir.AluOpType.add)
            sub0b = small.tile([128, 1], F32, tag="sub0b")
            nc.vector.tensor_add(sub0b, sub0f, biascol)

            # ---- transpose xqw2' -> xqw2t [128h, 4, 128q] ----
            ps_q = ps_tr.tile([128, 512], BF16, tag="tr")
            for k in range(HT):
                nc.tensor.transpose(
                    ps_q[:, 128 * k:128 * (k + 1)],
                    xqw2[:, 128 * k:128 * (k + 1)], ident)
            xqw2t = small.tile([128, HT, 128], BF16, tag="xqw2t")
            nc.any.tensor_copy(xqw2t, ps_q.rearrange("p (k q) -> p k q", k=HT))

            # ---- transpose xc -> xcT [128h, HT, C] ----
            xct_t = xct_pool.tile([128, HT, C], BF16, tag="xct")
            for k in range(HT):
                for half in range(2):
                    ps_x = ps_tr.tile([128, 512], BF16, tag="tr")
                    for i in range(4):
                        t = 4 * half + i
                        nc.tensor.transpose(
                            ps_x[:, 128 * i:128 * (i + 1)],
                            xc_t[:, t, 128 * k:128 * (k + 1)], ident)
                    nc.any.tensor_copy(
                        xct_t[:, k, 512 * half:512 * (half + 1)], ps_x)

            # ---- S^T chunks + exp -> E^T; rc via accum ----
            et_t = et_pool.tile([128, C], BF16, tag="et")
            rc2 = small.tile([128, 2], F32, tag="rc2")
            for n in range(N_CHUNKS):
                sl = slice(NC_CHUNK * n, NC_CHUNK * (n + 1))
                ps_S = ps_s.tile([128, 512], F32, tag="s")
                for k in range(HT):
                    nc.tensor.matmul(
                        ps_S, xqw2t[:, k, :], xct_t[:, k, sl],
                        start=(k == 0), stop=(k == HT - 1))
                nc.scalar.activation(
                    et_t[:, sl], ps_S, mybir.ActivationFunctionType.Exp,
                    bias=sub0b, accum_out=rc2[:, n:n + 1])
            rcsum = small.tile([128, 1], F32, tag="rcsum")
            nc.vector.tensor_add(rcsum, rc2[:, 0:1], rc2[:, 1:2])
            rcinv = small.tile([128, 1], F32, tag="rcinv")
            nc.vector.reciprocal(rcinv, rcsum)

            # ---- E (c-partitioned) via transposes; rq ----
            esb_t = esb_pool.tile([128, CT, 128], BF16, tag="esb")
            for n in range(N_CHUNKS):
                ps_e = ps_tr.tile([128, 512], BF16, tag="tr")
                for i in range(4):
                    j = 4 * n + i
                    nc.tensor.transpose(
                        ps_e[:, 128 * i:128 * (i + 1)],
                        et_t[:, 128 * j:128 * (j + 1)], ident)
                nc.any.tensor_copy(
                    esb_t[:, 4 * n:4 * (n + 1), :],
                    ps_e.rearrange("p (j q) -> p j q", j=4))
            rq = small.tile([128, CT], F32, tag="rq")
            nc.vector.tensor_reduce(
                rq, esb_t, axis=mybir.AxisListType.X, op=mybir.AluOpType.add)
            rqinv = small.tile([128, CT], F32, tag="rqinv")
            nc.vector.reciprocal(rqinv, rq)

            # ---- tmp = (E.T @ xc) * rcinv ----
            ps_t0 = ps_s.tile([128, 512], F32, tag="s")
            for t in range(CT):
                nc.tensor.matmul(ps_t0, esb_t[:, t, :], xc_t[:, t, :],
                                 start=(t == 0), stop=(t == CT - 1))
            tmp = small.tile([128, H], BF16, tag="tmp")
            nc.scalar.activation(tmp, ps_t0, mybir.ActivationFunctionType.Copy,
                                 scale=rcinv)

            # ---- per m: c2q | q2c into one 2-bank PSUM tile, single evac ----
            staged = stage.tile([128, CT, 2 * H], BF16, tag="out")
            for m in range(CT):
                ps_y = ps_o.tile([128, 2 * H], F32, tag="o")
                lhsT = et_t[:, 128 * m:128 * (m + 1)]
                nc.tensor.matmul(ps_y[:, 0:H], lhsT, xq_t, start=True, stop=True)
                nc.tensor.matmul(ps_y[:, H:2 * H], lhsT, tmp, start=True, stop=True)
                nc.any.tensor_scalar_mul(staged[:, m, :], ps_y, rqinv[:, m:m + 1])

            nc.sync.dma_start(
                out=c2q_d[b].rearrange("(t p) h -> p t h", p=128),
                in_=staged[:, :, 0:H])
            nc.sync.dma_start(
                out=q2c_d[b].rearrange("(t p) h -> p t h", p=128),
                in_=staged[:, :, H:2 * H])

    nc.finalize()
    return nc


_CACHED_NC = None


def make_in_maps(x_context, x_query, W0, W1, W2, bias):
    xc16 = np.ascontiguousarray(np.asarray(x_context, dtype=np.float32)).astype(BF16_NP)
    xq16 = np.ascontiguousarray(np.asarray(x_query, dtype=np.float32)).astype(BF16_NP)
    w0 = np.asarray(W0, dtype=np.float32).astype(BF16_NP)
    w1 = np.asarray(W1, dtype=np.float32).astype(BF16_NP)
    w2 = np.asarray(W2, dtype=np.float32).astype(BF16_NP)
    bias32 = np.asarray(bias, dtype=np.float32)

    in_maps = []
    for i in range(N_CORES):
        sl = slice(i * B_LOC, (i + 1) * B_LOC)
        in_maps.append({
            "xc": xc16[sl], "xq": xq16[sl],
            "W0": w0, "W1": w1, "W2": w2, "bias": bias32,
        })
    return in_maps


def gather_outputs(res):
    c2q = np.concatenate(
        [np.asarray(rm["c2q"]).astype(np.float32) for rm in res.results], axis=0)
    q2c = np.concatenate(
        [np.asarray(rm["q2c"]).astype(np.float32) for rm in res.results], axis=0)
    return c2q, q2c


def kernel(x_context, x_query, context_mask, query_mask, W0, W1, W2, bias):
    global _CACHED_NC
    if _CACHED_NC is None:
        _CACHED_NC = build_nc()
    nc = _CACHED_NC

    in_maps = make_in_maps(x_context, x_query, W0, W1, W2, bias)
    res = run_bass_kernel_spmd(nc, in_maps, core_ids=list(range(N_CORES)))
    return gather_outputs(res)
